# revision 2
# baseline (speedup 1.0000x reference)
"""ReLU-attention (AttentionMobile) Trainium2 Bass kernel.

Reference computation (fp32):
    q  = x @ Wq ; kv = x @ Wkv ; k = v = kv          (per batch, [S, D])
    per head h (Dh=64): A = relu(q_h k_h^T / sqrt(Dh)); o_h = A v_h
    out = concat_h(o_h) @ Wout + bout

Sharding: batch*heads across 8 cores — core c = (b, head-block j) with
b = c // 4, j = c % 4; each core owns 4 heads (256 cols of Wq/Wkv, 256 rows
of Wout) of one batch and computes a partial [S, D] output; host sums the 4
partials per batch and adds bout.

On-core dataflow (all matmuls bf16 with fp32 PSUM accumulation):
    xT [D, S] (host-pretransposed)  -> SBUF as 8 k-tiles [128, S]
    QT pair tiles  [128, S] = (Wq_pair).T @ x.T      (scale 1/8 folded into Wq)
    V   t-tiles    [128, 256] = x @ Wkv_slice
    KT pair tiles  [128, S] = PE-transpose of V tiles
    per head, per 512-col s-chunk:
        scoresT [128t, 512s] = KT_h.T @ QT_h   (K=64 contraction)
        AT = relu(scoresT) -> bf16
        OT [64, 512] += V_t.T @ AT_t  over 16 t-tiles
    out-proj: partial[s, :] = OT.T @ Wout_slice  (K=256 over 2 pair tiles)
"""

import os
import sys

import numpy as np
import ml_dtypes

for _p in ("/opt/trn_rl_repo",):
    if os.path.isdir(_p) and _p not in sys.path:
        sys.path.insert(0, _p)

B, S, D = 2, 2048, 1024
HEADS, DH = 16, 64
N_CORES = 8
HPC = HEADS * B // N_CORES          # heads per core = 4
HC = HPC * DH                       # per-core head cols = 256
KT_N = D // 128                     # 8 contraction tiles for projections
TT_N = S // 128                     # 16 t-tiles
SC_W = 512                          # s-chunk width
SC_N = S // SC_W                    # 4 s-chunks

_CACHE = {}
LAST_RESULTS = None


def build_nc():
    """Build and compile the single-core SPMD Bass program."""
    import concourse.mybir as mybir
    import concourse.tile as tile
    from concourse import bacc
    from concourse.bass import ts, ds
    from concourse.masks import make_identity

    f32 = mybir.dt.float32
    bf16 = mybir.dt.bfloat16
    Relu = mybir.ActivationFunctionType.Relu
    Copy = mybir.ActivationFunctionType.Copy

    nc = bacc.Bacc("TRN2", target_bir_lowering=False, debug=False)

    xT_d = nc.dram_tensor("xT", (D, S), bf16, kind="ExternalInput")
    wq_d = nc.dram_tensor("wq", (D, HC), bf16, kind="ExternalInput")
    wkv_d = nc.dram_tensor("wkv", (D, HC), bf16, kind="ExternalInput")
    wout_d = nc.dram_tensor("wout", (HC, D), bf16, kind="ExternalInput")
    part_d = nc.dram_tensor("part", (S, D), f32, kind="ExternalOutput")

    with tile.TileContext(nc) as tc:
        with (
            tc.tile_pool(name="const", bufs=1) as cpool,
            tc.tile_pool(name="persist", bufs=1) as pp,
            tc.tile_pool(name="at", bufs=20) as atp,
            tc.tile_pool(name="osb", bufs=3) as outp,
            tc.tile_pool(name="psS", bufs=3, space="PSUM") as psS,
            tc.tile_pool(name="psM", bufs=3, space="PSUM") as psM,
            tc.tile_pool(name="psO", bufs=2, space="PSUM") as psO,
        ):
            ident = cpool.tile([128, 128], bf16)
            make_identity(nc, ident[:])

            xt = pp.tile([128, KT_N, S], bf16)       # x.T, d on partitions
            wq = pp.tile([128, KT_N, HC], bf16)
            wkv = pp.tile([128, KT_N, HC], bf16)
            wout = pp.tile([128, 2, D], bf16)
            qt = pp.tile([128, 2, S], bf16)          # per pair: [2 heads*64, S]
            kt = pp.tile([128, 2, S], bf16)
            vt = pp.tile([128, TT_N, HC], bf16)      # t on partitions
            ot = pp.tile([128, 2, S], bf16)          # attention out, dh on part

            for k in range(KT_N):
                nc.sync.dma_start(xt[:, k, :], xT_d[ts(k, 128), :])
                nc.sync.dma_start(wq[:, k, :], wq_d[ts(k, 128), :])
                nc.sync.dma_start(wkv[:, k, :], wkv_d[ts(k, 128), :])
            for p in range(2):
                nc.sync.dma_start(wout[:, p, :], wout_d[ts(p, 128), :])

            # ---- Q^T projection: per pair of heads, [128, S] ----
            for p in range(2):
                for sc in range(SC_N):
                    ps = psM.tile([128, SC_W], f32, tag="m")
                    for k in range(KT_N):
                        nc.tensor.matmul(
                            ps[:],
                            wq[:, k, ts(p, 128)],
                            xt[:, k, ds(sc * SC_W, SC_W)],
                            start=(k == 0),
                            stop=(k == KT_N - 1),
                        )
                    nc.scalar.activation(qt[:, p, ds(sc * SC_W, SC_W)], ps[:], Copy)

            # ---- V projection (natural layout) + K^T via PE transpose ----
            for tt in range(TT_N):
                ps = psM.tile([128, HC], f32, tag="m")
                for k in range(KT_N):
                    nc.tensor.matmul(
                        ps[:],
                        xt[:, k, ts(tt, 128)],
                        wkv[:, k, :],
                        start=(k == 0),
                        stop=(k == KT_N - 1),
                    )
                nc.vector.tensor_copy(vt[:, tt, :], ps[:])
                for p in range(2):
                    pst = psM.tile([128, 128], bf16, tag="m")
                    nc.tensor.transpose(pst[:], vt[:, tt, ts(p, 128)], ident[:])
                    nc.scalar.activation(kt[:, p, ts(tt, 128)], pst[:], Copy)

            # ---- attention + output projection, per s-chunk ----
            for sc in range(SC_N):
                s0 = sc * SC_W
                for h in range(HPC):
                    p, half = divmod(h, 2)
                    r0 = half * 64
                    ats = []
                    for tt in range(TT_N):
                        psa = psS.tile([128, SC_W], f32)
                        nc.tensor.matmul(
                            psa[:],
                            kt[r0 : r0 + 64, p, ts(tt, 128)],
                            qt[r0 : r0 + 64, p, ds(s0, SC_W)],
                            start=True,
                            stop=True,
                        )
                        at = atp.tile([128, SC_W], bf16, tag="at")
                        if tt % 2 == 0:
                            nc.scalar.activation(at[:], psa[:], Relu)
                        else:
                            nc.vector.tensor_scalar_max(at[:], psa[:], 0.0)
                        ats.append(at)
                    pso = psO.tile([64, SC_W], f32)
                    for tt in range(TT_N):
                        nc.tensor.matmul(
                            pso[:],
                            vt[:, tt, ds(h * DH, DH)],
                            ats[tt][:],
                            start=(tt == 0),
                            stop=(tt == TT_N - 1),
                        )
                    nc.vector.tensor_copy(ot[r0 : r0 + 64, p, ds(s0, SC_W)], pso[:])

                # out-proj for the 4 s-tiles of this chunk
                for st in range(4):
                    st0 = s0 + st * 128
                    osb = outp.tile([128, D], f32, tag="osb")
                    for nch in range(2):
                        psf = psM.tile([128, SC_W], f32, tag="m")
                        for p in range(2):
                            nc.tensor.matmul(
                                psf[:],
                                ot[:, p, ds(st0, 128)],
                                wout[:, p, ds(nch * SC_W, SC_W)],
                                start=(p == 0),
                                stop=(p == 1),
                            )
                        if nch == 0:
                            nc.scalar.activation(osb[:, ds(0, SC_W)], psf[:], Copy)
                        else:
                            nc.vector.tensor_copy(osb[:, ds(SC_W, SC_W)], psf[:])
                    nc.sync.dma_start(part_d[ds(st0, 128), :], osb[:])

    nc.compile()
    return nc


def _get_nc():
    if "nc" not in _CACHE:
        _CACHE["nc"] = build_nc()
    return _CACHE["nc"]


def make_in_maps(hidden_states, Wq, Wkv, Wout):
    bf = ml_dtypes.bfloat16
    x = np.asarray(hidden_states, dtype=np.float32)
    Wq = np.asarray(Wq, dtype=np.float32)
    Wkv = np.asarray(Wkv, dtype=np.float32)
    Wout = np.asarray(Wout, dtype=np.float32)
    scale = 1.0 / np.sqrt(np.float32(DH))
    xT = [np.ascontiguousarray(x[b].T).astype(bf) for b in range(B)]
    in_maps = []
    for c in range(N_CORES):
        b, j = divmod(c, N_CORES // B)
        h0 = j * HC
        in_maps.append(
            {
                "xT": xT[b],
                "wq": np.ascontiguousarray(Wq[:, h0 : h0 + HC] * scale).astype(bf),
                "wkv": np.ascontiguousarray(Wkv[:, h0 : h0 + HC]).astype(bf),
                "wout": np.ascontiguousarray(Wout[h0 : h0 + HC, :]).astype(bf),
            }
        )
    return in_maps


def kernel(**inputs):
    global LAST_RESULTS
    from concourse.bass_utils import run_bass_kernel_spmd

    nc = _get_nc()
    in_maps = make_in_maps(
        inputs["hidden_states"], inputs["Wq"], inputs["Wkv"], inputs["Wout"]
    )
    trace = bool(os.environ.get("KERNEL_TRACE"))
    res = run_bass_kernel_spmd(
        nc, in_maps, core_ids=list(range(N_CORES)), trace=trace
    )
    LAST_RESULTS = res
    out = np.zeros((B, S, D), dtype=np.float32)
    for c in range(N_CORES):
        out[c // (N_CORES // B)] += res.results[c]["part"]
    out += np.asarray(inputs["bout"], dtype=np.float32)[None, None, :]
    return out
